# revision 3
# baseline (speedup 1.0000x reference)
"""Space-to-depth (k=2) Trainium2 kernel.

Full op: in (32, 224, 224, 64) f32 -> out (32, 112, 112, 256) where
    out[b, oh, ow, kh*128 + kw*64 + c] = in[b, 2*oh+kh, 2*ow+kw, c]

Sharding: batch dim across 8 cores (4 images each).

Per-core kernel: pure data rearrangement done entirely with DRAM->DRAM DMA.
Key observation: for fixed kh, the input rows h = 2*oh+kh are read fully
contiguously (14336 f32 per row), and they scatter into the output as
512-byte chunks (128 f32 = the (kw, c) block for one ow) with stride 1024B.
Both access patterns are <=3 dims, so each kh is ONE InstDMACopy of 25.7MB:

  src  [[28672, 448], [1, 14336]]            offset kh*14336
  dst  [[28672, 448], [256, 112], [1, 128]]  offset kh*128

The two DMAs are issued on the two HWDGE rings (SP + ACT) in parallel.
"""

import numpy as np

N_CORES = 8
B_FULL = 32
B = B_FULL // N_CORES  # 4 images per core
H, W, C = 224, 224, 64
OH, OW = H // 2, W // 2
ROW = W * C            # 14336 elements per input row
OROW = OW * 4 * C      # 28672 elements per output row (== 2*ROW)
NROWPAIRS = B * OH     # 448 row-pairs per core

_cache = {}


def _build_nc():
    import concourse.bass as bass
    import concourse.mybir as mybir

    nc = bass.Bass()
    x = nc.declare_dram_parameter("x", [B, H, W, C], mybir.dt.float32, isOutput=False)
    y = nc.declare_dram_parameter(
        "y", [B, OH, OW, 4 * C], mybir.dt.float32, isOutput=True
    )

    with (
        nc.Block() as block,
        nc.semaphore("s0") as s0,
        nc.semaphore("s1") as s1,
    ):

        @block.sync
        def _(eng):
            eng.dma_start(
                out=bass.AP(y, 0, [[OROW, NROWPAIRS], [4 * C, OW], [1, 2 * C]]),
                in_=bass.AP(x, 0, [[2 * ROW, NROWPAIRS], [1, ROW]]),
            ).then_inc(s0, 16)
            eng.wait_ge(s0, 16)

        @block.scalar
        def _(eng):
            eng.dma_start(
                out=bass.AP(y, 2 * C, [[OROW, NROWPAIRS], [4 * C, OW], [1, 2 * C]]),
                in_=bass.AP(x, ROW, [[2 * ROW, NROWPAIRS], [1, ROW]]),
            ).then_inc(s1, 16)
            eng.wait_ge(s1, 16)

    return nc


def _get_runner():
    """Build (once) the jitted shard_map executor over 8 cores.

    Mirrors the multi-core path of bass2jax.run_bass_via_pjrt, but cached
    so repeated calls don't re-trace/re-compile.
    """
    if "runner" in _cache:
        return _cache["runner"]

    import jax
    import jax.numpy as jnp
    from jax.sharding import Mesh, NamedSharding, PartitionSpec
    from jax.experimental.shard_map import shard_map
    from concourse import bass2jax
    import concourse.mybir as mybir

    bass2jax.install_neuronx_cc_hook()
    nc = _build_nc()
    assert nc.dbg_addr is None
    partition_name = (
        nc.partition_id_tensor.name if nc.partition_id_tensor is not None else None
    )

    out_aval = jax.core.ShapedArray((B, OH, OW, 4 * C), np.float32)
    in_names = ("x", "y") + ((partition_name,) if partition_name else ())

    def _body(x, y_zero):
        operands = [x, y_zero]
        if partition_name:
            operands.append(bass2jax.partition_id_tensor())
        outs = bass2jax._bass_exec_p.bind(
            *operands,
            out_avals=(out_aval,),
            in_names=in_names,
            out_names=("y",),
            lowering_input_output_aliases=(),
            sim_require_finite=True,
            sim_require_nnan=True,
            nc=nc,
        )
        return outs[0]

    devices = jax.devices()[:N_CORES]
    assert len(devices) == N_CORES
    mesh = Mesh(np.asarray(devices), ("core",))
    sharding = NamedSharding(mesh, PartitionSpec("core"))
    sharded = jax.jit(
        shard_map(
            _body,
            mesh=mesh,
            in_specs=(PartitionSpec("core"), PartitionSpec("core")),
            out_specs=PartitionSpec("core"),
            check_rep=False,
        ),
        donate_argnums=(1,),
        keep_unused=True,
    )

    def run(x_dev, y_buf):
        return sharded(x_dev, y_buf)

    _cache["runner"] = (run, sharding)
    return _cache["runner"]


def _device_inputs(batch: np.ndarray):
    import jax

    run, sharding = _get_runner()
    x_dev = jax.device_put(np.ascontiguousarray(batch, dtype=np.float32), sharding)
    import jax.numpy as jnp

    y_buf = jax.device_put(
        np.zeros((B_FULL, OH, OW, 4 * C), np.float32), sharding
    )
    return run, x_dev, y_buf


def kernel(batch: np.ndarray) -> np.ndarray:
    batch = np.asarray(batch)
    assert batch.shape == (B_FULL, H, W, C), batch.shape
    run, x_dev, y_buf = _device_inputs(batch)
    out = run(x_dev, y_buf)
    return np.asarray(out)


def bench(batch: np.ndarray, iters: int = 32) -> float:
    """Steady-state per-iteration time in ns (async dispatch, single sync)."""
    import time
    import jax

    run, x_dev, y_buf = _device_inputs(batch)
    out = run(x_dev, y_buf)  # warmup + compile
    out = jax.block_until_ready(out)
    t0 = time.perf_counter()
    for _ in range(iters):
        out = run(x_dev, out)
    jax.block_until_ready(out)
    t1 = time.perf_counter()
    return (t1 - t0) / iters * 1e9


# revision 5
# speedup vs baseline: 1.3351x; 1.3351x over previous
"""Space-to-depth (k=2) Trainium2 kernel.

Full op: in (32, 224, 224, 64) f32 -> out (32, 112, 112, 256) where
    out[b, oh, ow, kh*128 + kw*64 + c] = in[b, 2*oh+kh, 2*ow+kw, c]

Sharding: batch dim across 8 cores (4 images each).

Per-core kernel: pure data rearrangement done entirely with DRAM->DRAM DMA,
one DMA per output row ("rowgather"): each output row (b, oh) is a fully
contiguous 112KB write, gathered from the two source rows 2*oh/2*oh+1 as
512-byte chunks (128 f32 = one ow's (kw, c) block, contiguous in both
layouts). Access patterns (f32 elements, per row-pair rp = b*112 + oh):

  dst  y[rp*28672 : +28672]   [[1, 28672]]               (contiguous)
  src  x @ rp*28672           [[128, 112], [14336, 2], [1, 128]]

448 DMAs per core, alternating between the two HWDGE rings (SP + ACT).
Measured (slope method, 8 cores): ~305 us/core = 337 GB/s HBM traffic,
within ~2% of the pure contiguous-copy roofline on the same fabric
(~298 us); strided 512B *reads* are hidden by HBM prefetch, whereas the
mirrored strided-write form costs ~10%.
"""

import numpy as np

N_CORES = 8
B_FULL = 32
B = B_FULL // N_CORES  # 4 images per core
H, W, C = 224, 224, 64
OH, OW = H // 2, W // 2
ROW = W * C            # 14336 elements per input row
OROW = OW * 4 * C      # 28672 elements per output row (== 2*ROW)
NROWPAIRS = B * OH     # 448 row-pairs per core

_cache = {}


def _build_nc():
    import concourse.bass as bass
    import concourse.mybir as mybir

    nc = bass.Bass()
    x = nc.declare_dram_parameter("x", [B, H, W, C], mybir.dt.float32, isOutput=False)
    y = nc.declare_dram_parameter(
        "y", [B, OH, OW, 4 * C], mybir.dt.float32, isOutput=True
    )

    with (
        nc.Block() as block,
        nc.semaphore("s0") as s0,
        nc.semaphore("s1") as s1,
    ):

        @block.sync
        def _(eng):
            for rp in range(0, NROWPAIRS, 2):
                eng.dma_start(
                    out=bass.AP(y, rp * OROW, [[1, OROW]]),
                    in_=bass.AP(x, rp * OROW, [[128, OW], [ROW, 2], [1, 2 * C]]),
                ).then_inc(s0, 16)
            eng.wait_ge(s0, 16 * NROWPAIRS // 2)

        @block.scalar
        def _(eng):
            for rp in range(1, NROWPAIRS, 2):
                eng.dma_start(
                    out=bass.AP(y, rp * OROW, [[1, OROW]]),
                    in_=bass.AP(x, rp * OROW, [[128, OW], [ROW, 2], [1, 2 * C]]),
                ).then_inc(s1, 16)
            eng.wait_ge(s1, 16 * NROWPAIRS // 2)

    return nc


def _get_runner():
    """Build (once) the jitted shard_map executor over 8 cores.

    Mirrors the multi-core path of bass2jax.run_bass_via_pjrt, but cached
    so repeated calls don't re-trace/re-compile.
    """
    if "runner" in _cache:
        return _cache["runner"]

    import jax
    import jax.numpy as jnp
    from jax.sharding import Mesh, NamedSharding, PartitionSpec
    from jax.experimental.shard_map import shard_map
    from concourse import bass2jax
    import concourse.mybir as mybir

    bass2jax.install_neuronx_cc_hook()
    nc = _build_nc()
    assert nc.dbg_addr is None
    partition_name = (
        nc.partition_id_tensor.name if nc.partition_id_tensor is not None else None
    )

    out_aval = jax.core.ShapedArray((B, OH, OW, 4 * C), np.float32)
    in_names = ("x", "y") + ((partition_name,) if partition_name else ())

    def _body(x, y_zero):
        operands = [x, y_zero]
        if partition_name:
            operands.append(bass2jax.partition_id_tensor())
        outs = bass2jax._bass_exec_p.bind(
            *operands,
            out_avals=(out_aval,),
            in_names=in_names,
            out_names=("y",),
            lowering_input_output_aliases=(),
            sim_require_finite=True,
            sim_require_nnan=True,
            nc=nc,
        )
        return outs[0]

    devices = jax.devices()[:N_CORES]
    assert len(devices) == N_CORES
    mesh = Mesh(np.asarray(devices), ("core",))
    sharding = NamedSharding(mesh, PartitionSpec("core"))
    sharded = jax.jit(
        shard_map(
            _body,
            mesh=mesh,
            in_specs=(PartitionSpec("core"), PartitionSpec("core")),
            out_specs=PartitionSpec("core"),
            check_rep=False,
        ),
        donate_argnums=(1,),
        keep_unused=True,
    )

    def run(x_dev, y_buf):
        return sharded(x_dev, y_buf)

    _cache["runner"] = (run, sharding)
    return _cache["runner"]


def _device_inputs(batch: np.ndarray):
    import jax

    run, sharding = _get_runner()
    x_dev = jax.device_put(np.ascontiguousarray(batch, dtype=np.float32), sharding)
    import jax.numpy as jnp

    y_buf = jax.device_put(
        np.zeros((B_FULL, OH, OW, 4 * C), np.float32), sharding
    )
    return run, x_dev, y_buf


def kernel(batch: np.ndarray) -> np.ndarray:
    batch = np.asarray(batch)
    assert batch.shape == (B_FULL, H, W, C), batch.shape
    run, x_dev, y_buf = _device_inputs(batch)
    out = run(x_dev, y_buf)
    return np.asarray(out)


def bench(batch: np.ndarray, iters: int = 32) -> float:
    """Steady-state per-iteration time in ns (async dispatch, single sync)."""
    import time
    import jax

    run, x_dev, y_buf = _device_inputs(batch)
    out = run(x_dev, y_buf)  # warmup + compile
    out = jax.block_until_ready(out)
    t0 = time.perf_counter()
    for _ in range(iters):
        out = run(x_dev, out)
    jax.block_until_ready(out)
    t1 = time.perf_counter()
    return (t1 - t0) / iters * 1e9


# revision 6
# speedup vs baseline: 14.2409x; 10.6666x over previous
"""Space-to-depth (k=2) Trainium2 kernel.

Full op: in (32, 224, 224, 64) f32 -> out (32, 112, 112, 256) where
    out[b, oh, ow, kh*128 + kw*64 + c] = in[b, 2*oh+kh, 2*ow+kw, c]

Sharding: batch dim across 8 cores (4 images each).

Per-core kernel: pure data rearrangement done entirely with DRAM->DRAM DMA,
one DMA per output row ("rowgather"): each output row (b, oh) is a fully
contiguous 112KB write, gathered from the two source rows 2*oh/2*oh+1 as
512-byte chunks (128 f32 = one ow's (kw, c) block, contiguous in both
layouts). Access patterns (f32 elements, per row-pair rp = b*112 + oh):

  dst  y[rp*28672 : +28672]   [[1, 28672]]               (contiguous)
  src  x @ rp*28672           [[128, 112], [14336, 2], [1, 128]]

448 DMAs per core, alternating between the two HWDGE rings (SP + ACT).
Measured (slope method over K-repeat NEFFs, 8 cores): ~305 us/core
= ~337 GB/s of HBM traffic per NeuronCore, within ~2% of the pure
contiguous-copy roofline measured on the same fabric (~298 us). Strided
512B *reads* are hidden by HBM prefetch; the mirrored strided-write form
(contiguous reads, 512B scattered writes) costs ~10% more.
"""

import numpy as np

N_CORES = 8
B_FULL = 32
B = B_FULL // N_CORES  # 4 images per core
H, W, C = 224, 224, 64
OH, OW = H // 2, W // 2
ROW = W * C            # 14336 elements per input row
OROW = OW * 4 * C      # 28672 elements per output row (== 2*ROW)
NROWPAIRS = B * OH     # 448 row-pairs per core

_cache = {}


def _build_nc():
    import concourse.bass as bass
    import concourse.mybir as mybir

    nc = bass.Bass()
    x = nc.declare_dram_parameter("x", [B, H, W, C], mybir.dt.float32, isOutput=False)
    y = nc.declare_dram_parameter(
        "y", [B, OH, OW, 4 * C], mybir.dt.float32, isOutput=True
    )

    with (
        nc.Block() as block,
        nc.semaphore("s0") as s0,
        nc.semaphore("s1") as s1,
    ):

        @block.sync
        def _(eng):
            for rp in range(0, NROWPAIRS, 2):
                eng.dma_start(
                    out=bass.AP(y, rp * OROW, [[1, OROW]]),
                    in_=bass.AP(x, rp * OROW, [[128, OW], [ROW, 2], [1, 2 * C]]),
                ).then_inc(s0, 16)
            eng.wait_ge(s0, 16 * NROWPAIRS // 2)

        @block.scalar
        def _(eng):
            for rp in range(1, NROWPAIRS, 2):
                eng.dma_start(
                    out=bass.AP(y, rp * OROW, [[1, OROW]]),
                    in_=bass.AP(x, rp * OROW, [[128, OW], [ROW, 2], [1, 2 * C]]),
                ).then_inc(s1, 16)
            eng.wait_ge(s1, 16 * NROWPAIRS // 2)

    return nc


def _get_runner():
    """Build (once) the jitted shard_map executor over 8 cores.

    Mirrors the multi-core path of bass2jax.run_bass_via_pjrt, but cached
    so repeated calls don't re-trace/re-compile.
    """
    if "runner" in _cache:
        return _cache["runner"]

    import jax
    from jax.sharding import Mesh, NamedSharding, PartitionSpec
    from jax.experimental.shard_map import shard_map
    from concourse import bass2jax

    bass2jax.install_neuronx_cc_hook()
    nc = _build_nc()
    assert nc.dbg_addr is None
    partition_name = (
        nc.partition_id_tensor.name if nc.partition_id_tensor is not None else None
    )

    out_aval = jax.core.ShapedArray((B, OH, OW, 4 * C), np.float32)
    in_names = ("x", "y") + ((partition_name,) if partition_name else ())

    def _body(x, y_zero):
        operands = [x, y_zero]
        if partition_name:
            operands.append(bass2jax.partition_id_tensor())
        outs = bass2jax._bass_exec_p.bind(
            *operands,
            out_avals=(out_aval,),
            in_names=in_names,
            out_names=("y",),
            lowering_input_output_aliases=(),
            sim_require_finite=True,
            sim_require_nnan=True,
            nc=nc,
        )
        return outs[0]

    devices = jax.devices()[:N_CORES]
    assert len(devices) == N_CORES
    mesh = Mesh(np.asarray(devices), ("core",))
    sharding = NamedSharding(mesh, PartitionSpec("core"))
    sharded = jax.jit(
        shard_map(
            _body,
            mesh=mesh,
            in_specs=(PartitionSpec("core"), PartitionSpec("core")),
            out_specs=PartitionSpec("core"),
            check_rep=False,
        ),
        donate_argnums=(1,),
        keep_unused=True,
    )

    _cache["runner"] = (sharded, sharding)
    return _cache["runner"]


def _kernel_axon(batch: np.ndarray) -> np.ndarray:
    import jax

    run, sharding = _get_runner()
    x_dev = jax.device_put(batch, sharding)
    y_buf = _cache.pop("ybuf", None)
    if y_buf is None:
        y_buf = jax.device_put(np.zeros((B_FULL, OH, OW, 4 * C), np.float32), sharding)
    out = run(x_dev, y_buf)
    res = np.asarray(out)
    # recycle the device output buffer as the next call's donated output
    _cache["ybuf"] = out
    return res


def _kernel_fallback(batch: np.ndarray) -> np.ndarray:
    """Native (non-axon) path: plain run_bass_kernel_spmd."""
    from concourse.bass_utils import run_bass_kernel_spmd

    if "nc" not in _cache:
        _cache["nc"] = _build_nc()
    core_ids = list(range(N_CORES))
    in_maps = [{"x": batch[i * B : (i + 1) * B]} for i in core_ids]
    res = run_bass_kernel_spmd(_cache["nc"], in_maps, core_ids)
    out = np.empty((B_FULL, OH, OW, 4 * C), dtype=np.float32)
    for i in core_ids:
        out[i * B : (i + 1) * B] = res.results[i]["y"].reshape(B, OH, OW, 4 * C)
    return out


def kernel(batch: np.ndarray) -> np.ndarray:
    batch = np.ascontiguousarray(np.asarray(batch, dtype=np.float32))
    assert batch.shape == (B_FULL, H, W, C), batch.shape

    try:
        from concourse._compat import axon_active
    except ImportError:
        axon_active = None

    if axon_active is not None and axon_active():
        return _kernel_axon(batch)
    return _kernel_fallback(batch)


# revision 7
# speedup vs baseline: 14.5472x; 1.0215x over previous
"""Space-to-depth (k=2) Trainium2 kernel.

Full op: in (32, 224, 224, 64) f32 -> out (32, 112, 112, 256) where
    out[b, oh, ow, kh*128 + kw*64 + c] = in[b, 2*oh+kh, 2*ow+kw, c]

Sharding: batch dim across 8 cores (4 images each).

Per-core kernel: pure data rearrangement done entirely with DRAM->DRAM DMA,
one DMA per output row ("rowgather"): each output row (b, oh) is a fully
contiguous 112KB write, gathered from the two source rows 2*oh/2*oh+1 as
512-byte chunks (128 f32 = one ow's (kw, c) block, contiguous in both
layouts). Access patterns (f32 elements, per row-pair rp = b*112 + oh):

  dst  y[rp*28672 : +28672]   [[1, 28672]]               (contiguous)
  src  x @ rp*28672           [[128, 112], [14336, 2], [1, 128]]

448 DMAs per core, alternating between the two HWDGE rings (SP + ACT).
Measured (slope method over K-repeat NEFFs, 8 cores): ~305 us/core
= ~337 GB/s of HBM traffic per NeuronCore, within ~2% of the pure
contiguous-copy roofline measured on the same fabric (~298 us). Strided
512B *reads* are hidden by HBM prefetch; the mirrored strided-write form
(contiguous reads, 512B scattered writes) costs ~10% more.
"""

import numpy as np

N_CORES = 8
B_FULL = 32
B = B_FULL // N_CORES  # 4 images per core
H, W, C = 224, 224, 64
OH, OW = H // 2, W // 2
ROW = W * C            # 14336 elements per input row
OROW = OW * 4 * C      # 28672 elements per output row (== 2*ROW)
NROWPAIRS = B * OH     # 448 row-pairs per core

_cache = {}


def _build_nc():
    import concourse.bass as bass
    import concourse.mybir as mybir

    nc = bass.Bass()
    x = nc.declare_dram_parameter("x", [B, H, W, C], mybir.dt.float32, isOutput=False)
    y = nc.declare_dram_parameter(
        "y", [B, OH, OW, 4 * C], mybir.dt.float32, isOutput=True
    )

    with (
        nc.Block() as block,
        nc.semaphore("s0") as s0,
        nc.semaphore("s1") as s1,
    ):

        @block.sync
        def _(eng):
            for rp in range(0, NROWPAIRS, 2):
                eng.dma_start(
                    out=bass.AP(y, rp * OROW, [[1, OROW]]),
                    in_=bass.AP(x, rp * OROW, [[128, OW], [ROW, 2], [1, 2 * C]]),
                ).then_inc(s0, 16)
            eng.wait_ge(s0, 16 * NROWPAIRS // 2)

        @block.scalar
        def _(eng):
            for rp in range(1, NROWPAIRS, 2):
                eng.dma_start(
                    out=bass.AP(y, rp * OROW, [[1, OROW]]),
                    in_=bass.AP(x, rp * OROW, [[128, OW], [ROW, 2], [1, 2 * C]]),
                ).then_inc(s1, 16)
            eng.wait_ge(s1, 16 * NROWPAIRS // 2)

    return nc


def _get_runner():
    """Build (once) the jitted shard_map executor over 8 cores.

    Mirrors the multi-core path of bass2jax.run_bass_via_pjrt, but cached
    so repeated calls don't re-trace/re-compile.
    """
    if "runner" in _cache:
        return _cache["runner"]

    import jax
    from jax.sharding import Mesh, NamedSharding, PartitionSpec
    from jax.experimental.shard_map import shard_map
    from concourse import bass2jax

    bass2jax.install_neuronx_cc_hook()
    nc = _build_nc()
    assert nc.dbg_addr is None
    partition_name = (
        nc.partition_id_tensor.name if nc.partition_id_tensor is not None else None
    )

    out_aval = jax.core.ShapedArray((B, OH, OW, 4 * C), np.float32)
    in_names = ("x", "y") + ((partition_name,) if partition_name else ())

    def _body(x, y_zero):
        operands = [x, y_zero]
        if partition_name:
            operands.append(bass2jax.partition_id_tensor())
        outs = bass2jax._bass_exec_p.bind(
            *operands,
            out_avals=(out_aval,),
            in_names=in_names,
            out_names=("y",),
            lowering_input_output_aliases=(),
            sim_require_finite=True,
            sim_require_nnan=True,
            nc=nc,
        )
        return outs[0]

    devices = jax.devices()[:N_CORES]
    assert len(devices) == N_CORES
    mesh = Mesh(np.asarray(devices), ("core",))
    sharding = NamedSharding(mesh, PartitionSpec("core"))
    sharded = jax.jit(
        shard_map(
            _body,
            mesh=mesh,
            in_specs=(PartitionSpec("core"), PartitionSpec("core")),
            out_specs=PartitionSpec("core"),
            check_rep=False,
        ),
        donate_argnums=(1,),
        keep_unused=True,
    )

    _cache["runner"] = (sharded, sharding)
    return _cache["runner"]


def _put_sharded(arr: np.ndarray, sharding):
    """Shard arr on axis 0 across the 8 cores with parallel per-shard
    transfers (~12x faster than a single jax.device_put through axon)."""
    import jax
    from concurrent.futures import ThreadPoolExecutor

    devices = list(sharding.mesh.devices.flat)
    n = len(devices)
    sb = arr.shape[0] // n
    shards = [arr[i * sb : (i + 1) * sb] for i in range(n)]
    with ThreadPoolExecutor(n) as ex:
        arrs = list(ex.map(lambda t: jax.device_put(t[0], t[1]), zip(shards, devices)))
    return jax.make_array_from_single_device_arrays(arr.shape, sharding, arrs)


def _kernel_axon(batch: np.ndarray) -> np.ndarray:
    run, sharding = _get_runner()
    x_dev = _put_sharded(batch, sharding)
    y_buf = _cache.pop("ybuf", None)
    if y_buf is None:
        y_buf = _put_sharded(np.zeros((B_FULL, OH, OW, 4 * C), np.float32), sharding)
    out = run(x_dev, y_buf)
    res = np.asarray(out)
    # recycle the device output buffer as the next call's donated output
    _cache["ybuf"] = out
    return res


def _kernel_fallback(batch: np.ndarray) -> np.ndarray:
    """Native (non-axon) path: plain run_bass_kernel_spmd."""
    from concourse.bass_utils import run_bass_kernel_spmd

    if "nc" not in _cache:
        _cache["nc"] = _build_nc()
    core_ids = list(range(N_CORES))
    in_maps = [{"x": batch[i * B : (i + 1) * B]} for i in core_ids]
    res = run_bass_kernel_spmd(_cache["nc"], in_maps, core_ids)
    out = np.empty((B_FULL, OH, OW, 4 * C), dtype=np.float32)
    for i in core_ids:
        out[i * B : (i + 1) * B] = res.results[i]["y"].reshape(B, OH, OW, 4 * C)
    return out


def kernel(batch: np.ndarray) -> np.ndarray:
    batch = np.ascontiguousarray(np.asarray(batch, dtype=np.float32))
    assert batch.shape == (B_FULL, H, W, C), batch.shape

    try:
        from concourse._compat import axon_active
    except ImportError:
        axon_active = None

    if axon_active is not None and axon_active():
        return _kernel_axon(batch)
    return _kernel_fallback(batch)


# revision 10
# speedup vs baseline: 14.8991x; 1.0242x over previous
"""Space-to-depth (k=2) Trainium2 kernel.

Full op: in (32, 224, 224, 64) f32 -> out (32, 112, 112, 256) where
    out[b, oh, ow, kh*128 + kw*64 + c] = in[b, 2*oh+kh, 2*ow+kw, c]

Sharding: batch dim across 8 cores (4 images each).

Per-core kernel: pure data rearrangement done entirely with DRAM->DRAM DMA,
one DMA per output row ("rowgather"): each output row (b, oh) is a fully
contiguous 112KB write, gathered from the two source rows 2*oh/2*oh+1 as
512-byte chunks (128 f32 = one ow's (kw, c) block, contiguous in both
layouts). Access patterns (f32 elements, per row-pair rp = b*112 + oh):

  dst  y[rp*28672 : +28672]   [[1, 28672]]               (contiguous)
  src  x @ rp*28672           [[128, 112], [14336, 2], [1, 128]]

448 DMAs per core, alternating between the two HWDGE rings (SP + ACT).
Measured (slope method over K-repeat NEFFs, 8 cores): ~305 us/core
= ~337 GB/s of HBM traffic per NeuronCore, within ~2% of the pure
contiguous-copy roofline measured on the same fabric (~298 us). Strided
512B *reads* are hidden by HBM prefetch; the mirrored strided-write form
(contiguous reads, 512B scattered writes) costs ~10% more.
"""

import numpy as np

N_CORES = 8
B_FULL = 32
B = B_FULL // N_CORES  # 4 images per core
H, W, C = 224, 224, 64
OH, OW = H // 2, W // 2
ROW = W * C            # 14336 elements per input row
OROW = OW * 4 * C      # 28672 elements per output row (== 2*ROW)
NROWPAIRS = B * OH     # 448 row-pairs per core

_cache = {}


def _build_nc():
    import concourse.bass as bass
    import concourse.mybir as mybir

    nc = bass.Bass()
    x = nc.declare_dram_parameter("x", [B, H, W, C], mybir.dt.float32, isOutput=False)
    y = nc.declare_dram_parameter(
        "y", [B, OH, OW, 4 * C], mybir.dt.float32, isOutput=True
    )

    with (
        nc.Block() as block,
        nc.semaphore("s0") as s0,
        nc.semaphore("s1") as s1,
    ):

        @block.sync
        def _(eng):
            for rp in range(0, NROWPAIRS, 2):
                eng.dma_start(
                    out=bass.AP(y, rp * OROW, [[1, OROW]]),
                    in_=bass.AP(x, rp * OROW, [[128, OW], [ROW, 2], [1, 2 * C]]),
                ).then_inc(s0, 16)
            eng.wait_ge(s0, 16 * NROWPAIRS // 2)

        @block.scalar
        def _(eng):
            for rp in range(1, NROWPAIRS, 2):
                eng.dma_start(
                    out=bass.AP(y, rp * OROW, [[1, OROW]]),
                    in_=bass.AP(x, rp * OROW, [[128, OW], [ROW, 2], [1, 2 * C]]),
                ).then_inc(s1, 16)
            eng.wait_ge(s1, 16 * NROWPAIRS // 2)

    return nc


def _get_runner():
    """Build (once) the jitted shard_map executor over 8 cores.

    Mirrors the multi-core path of bass2jax.run_bass_via_pjrt, but cached
    so repeated calls don't re-trace/re-compile.
    """
    if "runner" in _cache:
        return _cache["runner"]

    import jax
    from jax.sharding import Mesh, NamedSharding, PartitionSpec

    import inspect

    try:
        from jax import shard_map as _shard_map
    except ImportError:
        from jax.experimental.shard_map import shard_map as _shard_map
    _norep_kw = (
        {"check_vma": False}
        if "check_vma" in inspect.signature(_shard_map).parameters
        else {"check_rep": False}
    )
    from concourse import bass2jax

    bass2jax.install_neuronx_cc_hook()
    nc = _build_nc()
    assert nc.dbg_addr is None
    partition_name = (
        nc.partition_id_tensor.name if nc.partition_id_tensor is not None else None
    )

    out_aval = jax.core.ShapedArray((B, OH, OW, 4 * C), np.float32)
    in_names = ("x", "y") + ((partition_name,) if partition_name else ())

    def _body(x, y_zero):
        operands = [x, y_zero]
        if partition_name:
            operands.append(bass2jax.partition_id_tensor())
        outs = bass2jax._bass_exec_p.bind(
            *operands,
            out_avals=(out_aval,),
            in_names=in_names,
            out_names=("y",),
            lowering_input_output_aliases=(),
            sim_require_finite=True,
            sim_require_nnan=True,
            nc=nc,
        )
        return outs[0]

    devices = jax.devices()[:N_CORES]
    assert len(devices) == N_CORES
    mesh = Mesh(np.asarray(devices), ("core",))
    sharding = NamedSharding(mesh, PartitionSpec("core"))
    sharded = jax.jit(
        _shard_map(
            _body,
            mesh=mesh,
            in_specs=(PartitionSpec("core"), PartitionSpec("core")),
            out_specs=PartitionSpec("core"),
            **_norep_kw,
        ),
        donate_argnums=(1,),
        keep_unused=True,
    )

    _cache["runner"] = (sharded, sharding)
    return _cache["runner"]


def _put_sharded(arr: np.ndarray, sharding):
    """Shard arr on axis 0 across the 8 cores with parallel per-shard
    transfers (~12x faster than a single jax.device_put through axon)."""
    import jax
    from concurrent.futures import ThreadPoolExecutor

    devices = list(sharding.mesh.devices.flat)
    n = len(devices)
    sb = arr.shape[0] // n
    shards = [arr[i * sb : (i + 1) * sb] for i in range(n)]
    with ThreadPoolExecutor(n) as ex:
        arrs = list(ex.map(lambda t: jax.device_put(t[0], t[1]), zip(shards, devices)))
    return jax.make_array_from_single_device_arrays(arr.shape, sharding, arrs)


def _kernel_axon(batch: np.ndarray) -> np.ndarray:
    run, sharding = _get_runner()
    x_dev = _put_sharded(batch, sharding)
    y_buf = _cache.pop("ybuf", None)
    if y_buf is None:
        y_buf = _put_sharded(np.zeros((B_FULL, OH, OW, 4 * C), np.float32), sharding)
    out = run(x_dev, y_buf)
    res = np.asarray(out)
    # recycle the device output buffer as the next call's donated output
    _cache["ybuf"] = out
    return res


def _kernel_fallback(batch: np.ndarray) -> np.ndarray:
    """Native (non-axon) path: plain run_bass_kernel_spmd."""
    from concourse.bass_utils import run_bass_kernel_spmd

    if "nc" not in _cache:
        _cache["nc"] = _build_nc()
    core_ids = list(range(N_CORES))
    in_maps = [{"x": batch[i * B : (i + 1) * B]} for i in core_ids]
    res = run_bass_kernel_spmd(_cache["nc"], in_maps, core_ids)
    out = np.empty((B_FULL, OH, OW, 4 * C), dtype=np.float32)
    for i in core_ids:
        out[i * B : (i + 1) * B] = res.results[i]["y"].reshape(B, OH, OW, 4 * C)
    return out


def kernel(batch: np.ndarray) -> np.ndarray:
    batch = np.ascontiguousarray(np.asarray(batch, dtype=np.float32))
    assert batch.shape == (B_FULL, H, W, C), batch.shape

    try:
        from concourse._compat import axon_active
    except ImportError:
        axon_active = None

    if axon_active is not None and axon_active():
        return _kernel_axon(batch)
    return _kernel_fallback(batch)
